# revision 44
# baseline (speedup 1.0000x reference)
"""Trainium2 Bass kernel for the smoothed Preisach hysteresis model.

Math: the reference per-step update
    s' = where(h_t > h_{t-1}, s + (1-s)*sigmoid((h_t-alpha)/temp),
                              s + (-1-s)*sigmoid((beta-h_t)/temp))
is a first-order linear recurrence s' = (1-g)s + sigma*g. With
u = (s+1)/2, the up-mask M_t (1 if h rose, else 0), z = M - u and
dM_t = M_t - M_{t-1}:

    z' = (z + dM_t) * a_t,   a_t = sigmoid(-arg_t)

dM is a host-known constant row in {-1,0,1}; arg[n,t] = p_t +
alpha_n*q_t + beta_n*r_t comes from a K=6 fp16 PE matmul (hi/lo fp16
splits for fp32-class accuracy), a_t from one ScalarE sigmoid pass, and
the recurrence itself is the DVE tensor_tensor_scan (~2.3 ns/step, the
hardware bottleneck).

Structure per core: 5 hysteron tiles of 128 partitions (640 rows/core,
8*640 = 5120; the 31 leftover hysterons run on the host in numpy).
Per tile: K=6 arg matmuls in 512-col chunks -> sigmoid passes (1024
cols) -> chained scans (2048-col chunks; tile 4 uses 1024 so the tail
stays small). The density readout (d^T z, K=128 matmuls accumulated in
a [1,512] PSUM bank, one ACT copy per chunk) runs in two passes: tiles
{0,1,2} as tile 2's chunks complete (hiding under tiles 3-4's scans)
and tiles {3,4} as tile 4's chunks complete, into two DRAM rows the
host sums. The DVE scan is the critical path (~2.35 ns/step in situ,
~48 us/core); ACT (~26 us) and PE (~19 us) hide under it.
"""

import sys

import numpy as np

sys.path.insert(0, "/opt/trn_rl_repo")

N = 5151
T = 4096
TEMP = 0.01
NCORES = 8
P = 128
TILES = 5                 # per-core hysteron tiles on device
NPC = TILES * P           # 640 hysterons per core; 8*640 = 5120
NDEV = NCORES * NPC       # 5120 device hysterons; rest on host
ROWS = TILES * P
K6 = 6                    # arg matmul contraction: a_hi,a_lo,b_hi,b_lo,1,1
CH = 512                  # matmul chunk along T (one PSUM bank fp32)
SCH = 1024                # scan chunk (chained via initial)
ACH = 1024                # ACT/matmul chunk (2 PSUM banks per arg tile)
NCH = T // CH

_PROG_CACHE = {}


def _build_program(reps=1, loop_n=0, skip=()):
    import contextlib

    import concourse.bass as bass
    import concourse.tile as tile
    from concourse import bacc, mybir

    f32 = mybir.dt.float32
    f16 = mybir.dt.float16
    f8 = mybir.dt.float8e4
    nc = bacc.Bacc("TRN2", target_bir_lowering=False, debug=False,
                   num_devices=NCORES)

    wt_d = nc.dram_tensor("wt", [K6, ROWS], f16, kind="ExternalInput")
    v_d = nc.dram_tensor("v", [K6, T], f16, kind="ExternalInput")
    dm_d = nc.dram_tensor("dm", [T], f16, kind="ExternalInput")
    dens_d = nc.dram_tensor("dens", [P, TILES], f16, kind="ExternalInput")
    # tiles 0-3 scan in 2048 chunks (better scan rate); tile 4 in 1024
    # chunks so the tail readout granularity stays small
    tile_sch = [2048, 2048, 2048, 2048, 1024]
    for tok in skip:
        if tok.startswith("sch"):
            tile_sch = [int(tok[3:])] * TILES

    ngroups = 2
    mpart_d = nc.dram_tensor("mpart", [ngroups, T], f32,
                             kind="ExternalOutput")

    wt_ap = wt_d.ap()
    v_ap = v_d.ap()
    dm_ap = dm_d.ap()
    dens_ap = dens_d.ap()
    mpart_ap = mpart_d.ap()

    ts = bass.ts
    Sigmoid = mybir.ActivationFunctionType.Sigmoid
    mult = mybir.AluOpType.mult
    add = mybir.AluOpType.add

    with tile.TileContext(nc) as tc:
        from contextlib import ExitStack
        with ExitStack() as ctx:
            consts = ctx.enter_context(tc.tile_pool(name="consts", bufs=1))
            apool = ctx.enter_context(tc.tile_pool(name="a", bufs=4))
            spool = ctx.enter_context(tc.tile_pool(name="s", bufs=2))
            mpool = ctx.enter_context(tc.tile_pool(name="m", bufs=2))
            ps_arg = ctx.enter_context(
                tc.tile_pool(name="ps_arg", bufs=3, space="PSUM"))
            ps_m = ctx.enter_context(
                tc.tile_pool(name="ps_m", bufs=1, space="PSUM"))

            wt_sb = consts.tile([K6, ROWS], f16)
            v_sb = consts.tile([K6, T], f16)
            dens_sb = consts.tile([P, TILES], f16)
            dm_bc = consts.tile([P, T], f16)

            nc.sync.dma_start(out=wt_sb[:], in_=wt_ap[:, :])
            nc.sync.dma_start(out=v_sb[:], in_=v_ap[:, :])
            # broadcast dM row to all 128 partitions via 0-stride DMA,
            # chunked across queues so it doesn't serialize the pipeline
            for j in range(NCH):
                src = bass.AP(tensor=dm_ap.tensor,
                              offset=dm_ap.offset + j * CH,
                              ap=[[0, P], [1, CH]])
                nc.sync.dma_start(out=dm_bc[:, ts(j, CH)], in_=src)
            nc.sync.dma_start(out=dens_sb[:], in_=dens_ap[:, :])

            if loop_n:
                loop_cm = tc.For_i(
                    0, loop_n, 1,
                    hint_engines=(mybir.EngineType.PE,
                                  mybir.EngineType.Activation,
                                  mybir.EngineType.DVE))
            else:
                loop_cm = contextlib.nullcontext()
            with loop_cm:
              for _rep in range(reps):
                m_rows = [mpool.tile([1, T], f32, name=f"mrow{g}")
                          for g in range(ngroups)]
                s_tiles = [None] * TILES

                def readout(rows, c, g, sch):
                    # density dot accumulated over a tile group for scan
                    # chunk c: [1, CH] matmuls start/stop-chained per CH
                    # sub-chunk into a 2-bank [1, 2*CH] PSUM tile, one
                    # ACT copy per 2 sub-chunks; host sums the group rows
                    for jj2 in range(sch // (2 * CH)):
                        mp = ps_m.tile([1, 2 * CH], f32, tag="mp")
                        for half in range(2):
                            j = c * (sch // CH) + jj2 * 2 + half
                            for k, i_r in enumerate(rows):
                                nc.tensor.matmul(
                                    out=mp[:, ts(half, CH)],
                                    lhsT=dens_sb[:, i_r:i_r + 1],
                                    rhs=s_tiles[i_r][:, ts(j, CH)],
                                    start=(k == 0),
                                    stop=(k == len(rows) - 1),
                                )
                        jo = c * sch + jj2 * 2 * CH
                        nc.scalar.copy(
                            out=m_rows[g][:, jo:jo + 2 * CH], in_=mp[:])
                    span = c * sch
                    nc.sync.dma_start(
                        out=mpart_ap[g:g + 1, span:span + sch],
                        in_=m_rows[g][:, span:span + sch])

                # emission schedule: tile pairs (0,1) and (2,3) alternate
                # scan chunks so each chain link's initial-column
                # read-back hides behind the partner tile's scan
                sched = []
                for i0 in (0, 2):
                    for c in range(T // tile_sch[i0]):
                        sched.append((i0, c))
                        sched.append((i0 + 1, c))
                sched += [(4, c) for c in range(T // tile_sch[4])]

                for i, c in sched:
                    sch = tile_sch[i]
                    nsch = T // sch
                    ach = min(ACH, sch)
                    if c == 0:
                        s_tiles[i] = spool.tile([P, T], f16,
                                                name=f"s{i}")
                    s = s_tiles[i]
                    if True:
                        if "arg" not in skip:
                            a = apool.tile([P, sch], f16)
                            for aj in range(sch // ach):
                                arg = ps_arg.tile([P, ach], f32,
                                                  tag="arg")
                                for jj in range(ach // CH):
                                    j = (c * sch + aj * ach) // CH + jj
                                    nc.tensor.matmul(
                                        out=arg[:, ts(jj, CH)],
                                        lhsT=wt_sb[:, ts(i, P)],
                                        rhs=v_sb[:, ts(j, CH)],
                                        start=True, stop=True,
                                    )
                                # a = sigmoid(-arg)
                                nc.scalar.activation(
                                    out=a[:, ts(aj, ach)], in_=arg[:],
                                    func=Sigmoid, scale=-1.0)
                            a_ap = a[:]
                        else:
                            a_ap = dm_bc[:, ts(c, sch)]
                        if "scan" not in skip:
                            init = (0.0 if c == 0
                                    else s[:, c * sch - 1:c * sch])
                            # z' = (z + dM) * a
                            nc.vector.tensor_tensor_scan(
                                out=s[:, ts(c, sch)],
                                data0=dm_bc[:, ts(c, sch)],
                                data1=a_ap,
                                initial=init, op0=add, op1=mult,
                            )
                        else:
                            nc.vector.tensor_copy(out=s[:, ts(c, sch)],
                                                  in_=a_ap)
                        if i == 2 and "readout" not in skip:
                            readout([0, 1, 2], c, 0, sch)
                        elif i == TILES - 1:
                            rows = ([i] if "readout" in skip else [3, 4])
                            readout(rows, c, 1, sch)
    nc.compile()
    return nc


def _split16(x):
    hi = x.astype(np.float16)
    lo = (x - hi.astype(np.float64)).astype(np.float16)
    return hi, lo


def _host_prep(h, mesh_points, raw_density):
    h = np.asarray(h, np.float32)
    mesh = np.asarray(mesh_points, np.float32)
    rd = np.asarray(raw_density, np.float32)
    beta = mesh[:, 0].astype(np.float64)
    alpha = mesh[:, 1].astype(np.float64)

    hprev = np.concatenate([[np.float32(0.0)], h[:-1]])
    up = h > hprev
    R = np.float64(1.0) / np.float64(np.float32(TEMP))
    h64 = h.astype(np.float64)
    q = np.where(up, -R, 0.0)
    r = np.where(up, 0.0, R)
    p = np.where(up, R * h64, -R * h64)
    p_hi, p_lo = _split16(p)
    q16 = q.astype(np.float16)
    r16 = r.astype(np.float16)
    V6 = np.stack([q16, q16, r16, r16, p_hi, p_lo]).astype(np.float16)

    M = up.astype(np.float64)                 # M_t in {0,1}
    Mprev = np.concatenate([[0.0], M[:-1]])
    dM = (M - Mprev).astype(np.float16)       # in {-1,0,1}

    dens = (1.0 / (1.0 + np.exp(-rd.astype(np.float64))))  # [N] float64

    in_maps = []
    d16sum = 0.0
    for c in range(NCORES):
        sl = slice(c * NPC, (c + 1) * NPC)
        a_c = alpha[sl]
        b_c = beta[sl]
        d_c = dens[sl]
        ah, al = _split16(a_c)
        bh, bl = _split16(b_c)
        wt = np.stack([ah, al, bh, bl,
                       np.ones(ROWS, np.float16),
                       np.ones(ROWS, np.float16)]).astype(np.float16)
        dens16 = d_c.astype(np.float16)
        dens_tiles = dens16.reshape(TILES, P).T  # [P, TILES]
        d16sum += dens16.astype(np.float64).sum()
        in_maps.append({
            "wt": wt,
            "v": V6,
            "dm": dM,
            "dens": dens_tiles,
        })
    return in_maps, dens, h, d16sum, M


def _host_tail_states(h, mesh_points):
    """Exact recurrence for the hysterons left off the device."""
    mesh = np.asarray(mesh_points, np.float64)
    beta = mesh[NDEV:, 0]
    alpha = mesh[NDEV:, 1]
    h32 = np.asarray(h, np.float32)
    n = alpha.shape[0]
    s = -np.ones(n)
    f = 0.0
    out = np.empty((T, n))
    R = 1.0 / np.float64(np.float32(TEMP))
    for t in range(T):
        ht = float(h32[t])
        if ht > f:
            g = 1.0 / (1.0 + np.exp(-(ht - alpha) * R))
            s = s + (1.0 - s) * g
        else:
            g = 1.0 / (1.0 + np.exp(-(beta - ht) * R))
            s = s + (-1.0 - s) * g
        f = ht
        out[t] = s
    return out  # [T, n]


def kernel(h, mesh_points, raw_density, raw_offset, raw_scale, raw_slope):
    from concourse.bass_utils import run_bass_kernel_spmd

    in_maps, dens, h32, d16sum, M = _host_prep(h, mesh_points, raw_density)

    if "prog" not in _PROG_CACHE:
        _PROG_CACHE["prog"] = _build_program()
    nc = _PROG_CACHE["prog"]

    res = run_bass_kernel_spmd(nc, in_maps, list(range(NCORES)))
    zpart = np.zeros(T, np.float64)
    for c in range(NCORES):
        zpart += (res.results[c]["mpart"].astype(np.float64)
                  .sum(axis=0).reshape(T))

    # host tail: the 31 hysterons beyond 8*640
    s_tail = _host_tail_states(h, mesh_points)          # [T, 31]
    num_tail = s_tail @ dens[NDEV:]                     # [T]

    def sigm(x):
        return 1.0 / (1.0 + np.exp(-np.float64(np.asarray(x, np.float32)[0])))

    offset = -10.0 + 20.0 * sigm(raw_offset)
    scale = 20.0 * sigm(raw_scale)
    slope = -20.0 + 40.0 * sigm(raw_slope)

    # s = 2u-1, u = M - z  =>  sum(d*s) = d16sum*(2M-1) - 2*sum(d*z)
    num_dev = d16sum * (2.0 * M - 1.0) - 2.0 * zpart
    m = (num_dev + num_tail) / dens.sum()
    out = scale * m + h32.astype(np.float64) * slope + offset
    return out.astype(np.float32)


# revision 45
# speedup vs baseline: 1.0340x; 1.0340x over previous
"""Trainium2 Bass kernel for the smoothed Preisach hysteresis model.

Math: the reference per-step update
    s' = where(h_t > h_{t-1}, s + (1-s)*sigmoid((h_t-alpha)/temp),
                              s + (-1-s)*sigmoid((beta-h_t)/temp))
is a first-order linear recurrence s' = (1-g)s + sigma*g. With
u = (s+1)/2, the up-mask M_t (1 if h rose, else 0), z = M - u and
dM_t = M_t - M_{t-1}:

    z' = (z + dM_t) * a_t,   a_t = sigmoid(-arg_t)

dM is a host-known constant row in {-1,0,1}; arg[n,t] = p_t +
alpha_n*q_t + beta_n*r_t comes from a K=6 fp16 PE matmul (hi/lo fp16
splits for fp32-class accuracy), a_t from one ScalarE sigmoid pass, and
the recurrence itself is the DVE tensor_tensor_scan (~2.3 ns/step, the
hardware bottleneck).

Structure per core: 5 hysteron tiles of 128 partitions (640 rows/core,
8*640 = 5120; the 31 leftover hysterons run on the host in numpy).
Per tile: K=6 arg matmuls in 512-col chunks -> sigmoid passes (1024
cols) -> chained scans (2048-col chunks; tile 4 uses 1024 so the tail
stays small). The density readout (d^T z, K=128 matmuls accumulated in
a [1,512] PSUM bank, one ACT copy per chunk) runs in two passes: tiles
{0,1,2} as tile 2's chunks complete (hiding under tiles 3-4's scans)
and tiles {3,4} as tile 4's chunks complete, into two DRAM rows the
host sums. The DVE scan is the critical path (~2.35 ns/step in situ,
~48 us/core); ACT (~26 us) and PE (~19 us) hide under it.
"""

import sys

import numpy as np

sys.path.insert(0, "/opt/trn_rl_repo")

N = 5151
T = 4096
TEMP = 0.01
NCORES = 8
P = 128
TILES = 5                 # per-core hysteron tiles on device
NPC = TILES * P           # 640 hysterons per core; 8*640 = 5120
NDEV = NCORES * NPC       # 5120 device hysterons; rest on host
ROWS = TILES * P
K6 = 6                    # arg matmul contraction: a_hi,a_lo,b_hi,b_lo,1,1
CH = 512                  # matmul chunk along T (one PSUM bank fp32)
SCH = 1024                # scan chunk (chained via initial)
ACH = 1024                # ACT/matmul chunk (2 PSUM banks per arg tile)
NCH = T // CH

_PROG_CACHE = {}


def _build_program(reps=1, loop_n=0, skip=()):
    import contextlib

    import concourse.bass as bass
    import concourse.tile as tile
    from concourse import bacc, mybir

    f32 = mybir.dt.float32
    f16 = mybir.dt.float16
    f8 = mybir.dt.float8e4
    nc = bacc.Bacc("TRN2", target_bir_lowering=False, debug=False,
                   num_devices=NCORES)

    wt_d = nc.dram_tensor("wt", [K6, ROWS], f16, kind="ExternalInput")
    v_d = nc.dram_tensor("v", [K6, T], f16, kind="ExternalInput")
    dm_d = nc.dram_tensor("dm", [T], f16, kind="ExternalInput")
    dens_d = nc.dram_tensor("dens", [P, TILES], f16, kind="ExternalInput")
    # tiles 0-3 scan in 2048 chunks (better scan rate); tile 4 in 1024
    # chunks so the tail readout granularity stays small
    tile_sch = [2048, 2048, 2048, 2048, 1024]
    for tok in skip:
        if tok.startswith("sch"):
            tile_sch = [int(tok[3:])] * TILES

    ngroups = 2
    mpart_d = nc.dram_tensor("mpart", [ngroups, T], f32,
                             kind="ExternalOutput")

    wt_ap = wt_d.ap()
    v_ap = v_d.ap()
    dm_ap = dm_d.ap()
    dens_ap = dens_d.ap()
    mpart_ap = mpart_d.ap()

    ts = bass.ts
    Sigmoid = mybir.ActivationFunctionType.Sigmoid
    mult = mybir.AluOpType.mult
    add = mybir.AluOpType.add

    with tile.TileContext(nc) as tc:
        from contextlib import ExitStack
        with ExitStack() as ctx:
            consts = ctx.enter_context(tc.tile_pool(name="consts", bufs=1))
            apool = ctx.enter_context(tc.tile_pool(name="a", bufs=4))
            spool = ctx.enter_context(tc.tile_pool(name="s", bufs=2))
            mpool = ctx.enter_context(tc.tile_pool(name="m", bufs=2))
            ps_arg = ctx.enter_context(
                tc.tile_pool(name="ps_arg", bufs=3, space="PSUM"))
            ps_m = ctx.enter_context(
                tc.tile_pool(name="ps_m", bufs=2, space="PSUM"))

            wt_sb = consts.tile([K6, ROWS], f16)
            v_sb = consts.tile([K6, T], f16)
            dens_sb = consts.tile([P, TILES], f16)
            dm_bc = consts.tile([P, T], f16)

            nc.sync.dma_start(out=wt_sb[:], in_=wt_ap[:, :])
            nc.sync.dma_start(out=v_sb[:], in_=v_ap[:, :])
            # broadcast dM row to all 128 partitions via 0-stride DMA,
            # chunked across queues so it doesn't serialize the pipeline
            for j in range(NCH):
                src = bass.AP(tensor=dm_ap.tensor,
                              offset=dm_ap.offset + j * CH,
                              ap=[[0, P], [1, CH]])
                nc.sync.dma_start(out=dm_bc[:, ts(j, CH)], in_=src)
            nc.sync.dma_start(out=dens_sb[:], in_=dens_ap[:, :])

            if loop_n:
                loop_cm = tc.For_i(
                    0, loop_n, 1,
                    hint_engines=(mybir.EngineType.PE,
                                  mybir.EngineType.Activation,
                                  mybir.EngineType.DVE))
            else:
                loop_cm = contextlib.nullcontext()
            with loop_cm:
              for _rep in range(reps):
                m_rows = [mpool.tile([1, T], f32, name=f"mrow{g}")
                          for g in range(ngroups)]
                s_tiles = [None] * TILES

                def readout(rows, c, g, sch):
                    # density dot accumulated over a tile group for scan
                    # chunk c: one [1, CH] PSUM bank per CH-chunk via
                    # matmul start/stop chaining, one ACT copy per chunk;
                    # the two group rows are summed on the host
                    for jj in range(sch // CH):
                        j = c * (sch // CH) + jj
                        mp = ps_m.tile([1, CH], f32, tag="mp")
                        for k, i_r in enumerate(rows):
                            nc.tensor.matmul(
                                out=mp[:],
                                lhsT=dens_sb[:, i_r:i_r + 1],
                                rhs=s_tiles[i_r][:, ts(j, CH)],
                                start=(k == 0), stop=(k == len(rows) - 1),
                            )
                        nc.scalar.copy(
                            out=m_rows[g][:, ts(j, CH)], in_=mp[:])
                    span = c * sch
                    nc.sync.dma_start(
                        out=mpart_ap[g:g + 1, span:span + sch],
                        in_=m_rows[g][:, span:span + sch])

                # emission schedule: tile pairs (0,1) and (2,3) alternate
                # scan chunks so each chain link's initial-column
                # read-back hides behind the partner tile's scan
                sched = []
                for i0 in (0, 2):
                    for c in range(T // tile_sch[i0]):
                        sched.append((i0, c))
                        sched.append((i0 + 1, c))
                sched += [(4, c) for c in range(T // tile_sch[4])]

                for i, c in sched:
                    sch = tile_sch[i]
                    nsch = T // sch
                    ach = min(ACH, sch)
                    if c == 0:
                        s_tiles[i] = spool.tile([P, T], f16,
                                                name=f"s{i}")
                    s = s_tiles[i]
                    if True:
                        if "arg" not in skip:
                            a = apool.tile([P, sch], f16)
                            for aj in range(sch // ach):
                                arg = ps_arg.tile([P, ach], f32,
                                                  tag="arg")
                                for jj in range(ach // CH):
                                    j = (c * sch + aj * ach) // CH + jj
                                    nc.tensor.matmul(
                                        out=arg[:, ts(jj, CH)],
                                        lhsT=wt_sb[:, ts(i, P)],
                                        rhs=v_sb[:, ts(j, CH)],
                                        start=True, stop=True,
                                    )
                                # a = sigmoid(-arg)
                                nc.scalar.activation(
                                    out=a[:, ts(aj, ach)], in_=arg[:],
                                    func=Sigmoid, scale=-1.0)
                            a_ap = a[:]
                        else:
                            a_ap = dm_bc[:, ts(c, sch)]
                        if "scan" not in skip:
                            init = (0.0 if c == 0
                                    else s[:, c * sch - 1:c * sch])
                            # z' = (z + dM) * a
                            nc.vector.tensor_tensor_scan(
                                out=s[:, ts(c, sch)],
                                data0=dm_bc[:, ts(c, sch)],
                                data1=a_ap,
                                initial=init, op0=add, op1=mult,
                            )
                        else:
                            nc.vector.tensor_copy(out=s[:, ts(c, sch)],
                                                  in_=a_ap)
                        if i == 2 and "readout" not in skip:
                            readout([0, 1, 2], c, 0, sch)
                        elif i == TILES - 1:
                            rows = ([i] if "readout" in skip else [3, 4])
                            readout(rows, c, 1, sch)
    nc.compile()
    return nc


def _split16(x):
    hi = x.astype(np.float16)
    lo = (x - hi.astype(np.float64)).astype(np.float16)
    return hi, lo


def _host_prep(h, mesh_points, raw_density):
    h = np.asarray(h, np.float32)
    mesh = np.asarray(mesh_points, np.float32)
    rd = np.asarray(raw_density, np.float32)
    beta = mesh[:, 0].astype(np.float64)
    alpha = mesh[:, 1].astype(np.float64)

    hprev = np.concatenate([[np.float32(0.0)], h[:-1]])
    up = h > hprev
    R = np.float64(1.0) / np.float64(np.float32(TEMP))
    h64 = h.astype(np.float64)
    q = np.where(up, -R, 0.0)
    r = np.where(up, 0.0, R)
    p = np.where(up, R * h64, -R * h64)
    p_hi, p_lo = _split16(p)
    q16 = q.astype(np.float16)
    r16 = r.astype(np.float16)
    V6 = np.stack([q16, q16, r16, r16, p_hi, p_lo]).astype(np.float16)

    M = up.astype(np.float64)                 # M_t in {0,1}
    Mprev = np.concatenate([[0.0], M[:-1]])
    dM = (M - Mprev).astype(np.float16)       # in {-1,0,1}

    dens = (1.0 / (1.0 + np.exp(-rd.astype(np.float64))))  # [N] float64

    in_maps = []
    d16sum = 0.0
    for c in range(NCORES):
        sl = slice(c * NPC, (c + 1) * NPC)
        a_c = alpha[sl]
        b_c = beta[sl]
        d_c = dens[sl]
        ah, al = _split16(a_c)
        bh, bl = _split16(b_c)
        wt = np.stack([ah, al, bh, bl,
                       np.ones(ROWS, np.float16),
                       np.ones(ROWS, np.float16)]).astype(np.float16)
        dens16 = d_c.astype(np.float16)
        dens_tiles = dens16.reshape(TILES, P).T  # [P, TILES]
        d16sum += dens16.astype(np.float64).sum()
        in_maps.append({
            "wt": wt,
            "v": V6,
            "dm": dM,
            "dens": dens_tiles,
        })
    return in_maps, dens, h, d16sum, M


def _host_tail_states(h, mesh_points):
    """Exact recurrence for the hysterons left off the device."""
    mesh = np.asarray(mesh_points, np.float64)
    beta = mesh[NDEV:, 0]
    alpha = mesh[NDEV:, 1]
    h32 = np.asarray(h, np.float32)
    n = alpha.shape[0]
    s = -np.ones(n)
    f = 0.0
    out = np.empty((T, n))
    R = 1.0 / np.float64(np.float32(TEMP))
    for t in range(T):
        ht = float(h32[t])
        if ht > f:
            g = 1.0 / (1.0 + np.exp(-(ht - alpha) * R))
            s = s + (1.0 - s) * g
        else:
            g = 1.0 / (1.0 + np.exp(-(beta - ht) * R))
            s = s + (-1.0 - s) * g
        f = ht
        out[t] = s
    return out  # [T, n]


def kernel(h, mesh_points, raw_density, raw_offset, raw_scale, raw_slope):
    from concourse.bass_utils import run_bass_kernel_spmd

    in_maps, dens, h32, d16sum, M = _host_prep(h, mesh_points, raw_density)

    if "prog" not in _PROG_CACHE:
        _PROG_CACHE["prog"] = _build_program()
    nc = _PROG_CACHE["prog"]

    res = run_bass_kernel_spmd(nc, in_maps, list(range(NCORES)))
    zpart = np.zeros(T, np.float64)
    for c in range(NCORES):
        zpart += (res.results[c]["mpart"].astype(np.float64)
                  .sum(axis=0).reshape(T))

    # host tail: the 31 hysterons beyond 8*640
    s_tail = _host_tail_states(h, mesh_points)          # [T, 31]
    num_tail = s_tail @ dens[NDEV:]                     # [T]

    def sigm(x):
        return 1.0 / (1.0 + np.exp(-np.float64(np.asarray(x, np.float32)[0])))

    offset = -10.0 + 20.0 * sigm(raw_offset)
    scale = 20.0 * sigm(raw_scale)
    slope = -20.0 + 40.0 * sigm(raw_slope)

    # s = 2u-1, u = M - z  =>  sum(d*s) = d16sum*(2M-1) - 2*sum(d*z)
    num_dev = d16sum * (2.0 * M - 1.0) - 2.0 * zpart
    m = (num_dev + num_tail) / dens.sum()
    out = scale * m + h32.astype(np.float64) * slope + offset
    return out.astype(np.float32)


# revision 46
# speedup vs baseline: 1.0554x; 1.0207x over previous
"""Trainium2 Bass kernel for the smoothed Preisach hysteresis model.

Math: the reference per-step update
    s' = where(h_t > h_{t-1}, s + (1-s)*sigmoid((h_t-alpha)/temp),
                              s + (-1-s)*sigmoid((beta-h_t)/temp))
is a first-order linear recurrence s' = (1-g)s + sigma*g. With
u = (s+1)/2, the up-mask M_t (1 if h rose, else 0), z = M - u and
dM_t = M_t - M_{t-1}:

    z' = (z + dM_t) * a_t,   a_t = sigmoid(-arg_t)

dM is a host-known constant row in {-1,0,1}; arg[n,t] = p_t +
alpha_n*q_t + beta_n*r_t comes from a K=6 fp16 PE matmul (hi/lo fp16
splits for fp32-class accuracy), a_t from one ScalarE sigmoid pass, and
the recurrence itself is the DVE tensor_tensor_scan (~2.3 ns/step, the
hardware bottleneck).

Structure per core: 5 hysteron tiles of 128 partitions (640 rows/core,
8*640 = 5120; the 31 leftover hysterons run on the host in numpy).
Per tile: K=6 arg matmuls in 512-col chunks -> sigmoid passes (1024
cols) -> chained scans (2048-col chunks; tile 4 uses 1024 so the tail
stays small). The density readout (d^T z, K=128 matmuls accumulated in
a [1,512] PSUM bank, one ACT copy per chunk) runs in two passes: tiles
{0,1,2} as tile 2's chunks complete (hiding under tiles 3-4's scans)
and tiles {3,4} as tile 4's chunks complete, into two DRAM rows the
host sums. The DVE scan is the critical path (~2.35 ns/step in situ,
~48 us/core); ACT (~26 us) and PE (~19 us) hide under it.
"""

import sys

import numpy as np

sys.path.insert(0, "/opt/trn_rl_repo")

N = 5151
T = 4096
TEMP = 0.01
NCORES = 8
P = 128
TILES = 5                 # per-core hysteron tiles on device
NPC = TILES * P           # 640 hysterons per core; 8*640 = 5120
NDEV = NCORES * NPC       # 5120 device hysterons; rest on host
ROWS = TILES * P
K6 = 6                    # arg matmul contraction: a_hi,a_lo,b_hi,b_lo,1,1
CH = 512                  # matmul chunk along T (one PSUM bank fp32)
SCH = 1024                # scan chunk (chained via initial)
ACH = 1024                # ACT/matmul chunk (2 PSUM banks per arg tile)
NCH = T // CH

_PROG_CACHE = {}


def _build_program(reps=1, loop_n=0, skip=()):
    import contextlib

    import concourse.bass as bass
    import concourse.tile as tile
    from concourse import bacc, mybir

    f32 = mybir.dt.float32
    f16 = mybir.dt.float16
    f8 = mybir.dt.float8e4
    nc = bacc.Bacc("TRN2", target_bir_lowering=False, debug=False,
                   num_devices=NCORES)

    wt_d = nc.dram_tensor("wt", [K6, ROWS], f16, kind="ExternalInput")
    v_d = nc.dram_tensor("v", [K6, T], f16, kind="ExternalInput")
    dm_d = nc.dram_tensor("dm", [T], f16, kind="ExternalInput")
    dens_d = nc.dram_tensor("dens", [P, TILES], f16, kind="ExternalInput")
    # tiles 0-3 scan in 2048 chunks (better scan rate); tile 4 in 1024
    # chunks so the tail readout granularity stays small
    tile_sch = [2048, 2048, 2048, 2048, 1024]
    for tok in skip:
        if tok.startswith("sch"):
            tile_sch = [int(tok[3:])] * TILES

    ngroups = 2
    mpart_d = nc.dram_tensor("mpart", [ngroups, T], f32,
                             kind="ExternalOutput")

    wt_ap = wt_d.ap()
    v_ap = v_d.ap()
    dm_ap = dm_d.ap()
    dens_ap = dens_d.ap()
    mpart_ap = mpart_d.ap()

    ts = bass.ts
    Sigmoid = mybir.ActivationFunctionType.Sigmoid
    mult = mybir.AluOpType.mult
    add = mybir.AluOpType.add

    with tile.TileContext(nc) as tc:
        from contextlib import ExitStack
        with ExitStack() as ctx:
            consts = ctx.enter_context(tc.tile_pool(name="consts", bufs=1))
            apool = ctx.enter_context(tc.tile_pool(name="a", bufs=4))
            spool = ctx.enter_context(tc.tile_pool(name="s", bufs=2))
            mpool = ctx.enter_context(tc.tile_pool(name="m", bufs=2))
            ps_arg = ctx.enter_context(
                tc.tile_pool(name="ps_arg", bufs=3, space="PSUM"))
            ps_m = ctx.enter_context(
                tc.tile_pool(name="ps_m", bufs=2, space="PSUM"))

            wt_sb = consts.tile([K6, ROWS], f16)
            v_sb = consts.tile([K6, T], f16)
            dens_sb = consts.tile([P, TILES], f16)
            dm_bc = consts.tile([P, T], f16)

            nc.sync.dma_start(out=wt_sb[:], in_=wt_ap[:, :])
            nc.sync.dma_start(out=v_sb[:], in_=v_ap[:, :])
            # broadcast dM row to all 128 partitions via 0-stride DMA,
            # chunked across queues so it doesn't serialize the pipeline
            for j in range(NCH):
                src = bass.AP(tensor=dm_ap.tensor,
                              offset=dm_ap.offset + j * CH,
                              ap=[[0, P], [1, CH]])
                nc.sync.dma_start(out=dm_bc[:, ts(j, CH)], in_=src)
            nc.sync.dma_start(out=dens_sb[:], in_=dens_ap[:, :])

            if loop_n:
                loop_cm = tc.For_i(
                    0, loop_n, 1,
                    hint_engines=(mybir.EngineType.PE,
                                  mybir.EngineType.Activation,
                                  mybir.EngineType.DVE))
            else:
                loop_cm = contextlib.nullcontext()
            with loop_cm:
              for _rep in range(reps):
                m_rows = [mpool.tile([1, T], f32, name=f"mrow{g}")
                          for g in range(ngroups)]
                s_tiles = [None] * TILES

                def readout(rows, c, g, sch):
                    # density dot accumulated over a tile group for scan
                    # chunk c: one [1, CH] PSUM bank per CH-chunk via
                    # matmul start/stop chaining, one ACT copy per chunk;
                    # the two group rows are summed on the host
                    for jj in range(sch // CH):
                        j = c * (sch // CH) + jj
                        mp = ps_m.tile([1, CH], f32, tag="mp")
                        for k, i_r in enumerate(rows):
                            nc.tensor.matmul(
                                out=mp[:],
                                lhsT=dens_sb[:, i_r:i_r + 1],
                                rhs=s_tiles[i_r][:, ts(j, CH)],
                                start=(k == 0), stop=(k == len(rows) - 1),
                            )
                        nc.scalar.copy(
                            out=m_rows[g][:, ts(j, CH)], in_=mp[:])
                        # flush per CH-chunk: the final DMA then waits
                        # only on the last 512-col copy, not the span
                        nc.sync.dma_start(
                            out=mpart_ap[g:g + 1, ts(j, CH)],
                            in_=m_rows[g][:, ts(j, CH)])

                # emission schedule: tile pairs (0,1) and (2,3) alternate
                # scan chunks so each chain link's initial-column
                # read-back hides behind the partner tile's scan
                sched = []
                for i0 in (0, 2):
                    for c in range(T // tile_sch[i0]):
                        sched.append((i0, c))
                        sched.append((i0 + 1, c))
                sched += [(4, c) for c in range(T // tile_sch[4])]

                for i, c in sched:
                    sch = tile_sch[i]
                    nsch = T // sch
                    ach = min(ACH, sch)
                    if c == 0:
                        s_tiles[i] = spool.tile([P, T], f16,
                                                name=f"s{i}")
                    s = s_tiles[i]
                    if True:
                        if "arg" not in skip:
                            a = apool.tile([P, sch], f16)
                            for aj in range(sch // ach):
                                arg = ps_arg.tile([P, ach], f32,
                                                  tag="arg")
                                for jj in range(ach // CH):
                                    j = (c * sch + aj * ach) // CH + jj
                                    nc.tensor.matmul(
                                        out=arg[:, ts(jj, CH)],
                                        lhsT=wt_sb[:, ts(i, P)],
                                        rhs=v_sb[:, ts(j, CH)],
                                        start=True, stop=True,
                                    )
                                # a = sigmoid(-arg)
                                nc.scalar.activation(
                                    out=a[:, ts(aj, ach)], in_=arg[:],
                                    func=Sigmoid, scale=-1.0)
                            a_ap = a[:]
                        else:
                            a_ap = dm_bc[:, ts(c, sch)]
                        if "scan" not in skip:
                            init = (0.0 if c == 0
                                    else s[:, c * sch - 1:c * sch])
                            # z' = (z + dM) * a
                            nc.vector.tensor_tensor_scan(
                                out=s[:, ts(c, sch)],
                                data0=dm_bc[:, ts(c, sch)],
                                data1=a_ap,
                                initial=init, op0=add, op1=mult,
                            )
                        else:
                            nc.vector.tensor_copy(out=s[:, ts(c, sch)],
                                                  in_=a_ap)
                        if i == 2 and "readout" not in skip:
                            readout([0, 1, 2], c, 0, sch)
                        elif i == TILES - 1:
                            rows = ([i] if "readout" in skip else [3, 4])
                            readout(rows, c, 1, sch)
    nc.compile()
    return nc


def _split16(x):
    hi = x.astype(np.float16)
    lo = (x - hi.astype(np.float64)).astype(np.float16)
    return hi, lo


def _host_prep(h, mesh_points, raw_density):
    h = np.asarray(h, np.float32)
    mesh = np.asarray(mesh_points, np.float32)
    rd = np.asarray(raw_density, np.float32)
    beta = mesh[:, 0].astype(np.float64)
    alpha = mesh[:, 1].astype(np.float64)

    hprev = np.concatenate([[np.float32(0.0)], h[:-1]])
    up = h > hprev
    R = np.float64(1.0) / np.float64(np.float32(TEMP))
    h64 = h.astype(np.float64)
    q = np.where(up, -R, 0.0)
    r = np.where(up, 0.0, R)
    p = np.where(up, R * h64, -R * h64)
    p_hi, p_lo = _split16(p)
    q16 = q.astype(np.float16)
    r16 = r.astype(np.float16)
    V6 = np.stack([q16, q16, r16, r16, p_hi, p_lo]).astype(np.float16)

    M = up.astype(np.float64)                 # M_t in {0,1}
    Mprev = np.concatenate([[0.0], M[:-1]])
    dM = (M - Mprev).astype(np.float16)       # in {-1,0,1}

    dens = (1.0 / (1.0 + np.exp(-rd.astype(np.float64))))  # [N] float64

    in_maps = []
    d16sum = 0.0
    for c in range(NCORES):
        sl = slice(c * NPC, (c + 1) * NPC)
        a_c = alpha[sl]
        b_c = beta[sl]
        d_c = dens[sl]
        ah, al = _split16(a_c)
        bh, bl = _split16(b_c)
        wt = np.stack([ah, al, bh, bl,
                       np.ones(ROWS, np.float16),
                       np.ones(ROWS, np.float16)]).astype(np.float16)
        dens16 = d_c.astype(np.float16)
        dens_tiles = dens16.reshape(TILES, P).T  # [P, TILES]
        d16sum += dens16.astype(np.float64).sum()
        in_maps.append({
            "wt": wt,
            "v": V6,
            "dm": dM,
            "dens": dens_tiles,
        })
    return in_maps, dens, h, d16sum, M


def _host_tail_states(h, mesh_points):
    """Exact recurrence for the hysterons left off the device."""
    mesh = np.asarray(mesh_points, np.float64)
    beta = mesh[NDEV:, 0]
    alpha = mesh[NDEV:, 1]
    h32 = np.asarray(h, np.float32)
    n = alpha.shape[0]
    s = -np.ones(n)
    f = 0.0
    out = np.empty((T, n))
    R = 1.0 / np.float64(np.float32(TEMP))
    for t in range(T):
        ht = float(h32[t])
        if ht > f:
            g = 1.0 / (1.0 + np.exp(-(ht - alpha) * R))
            s = s + (1.0 - s) * g
        else:
            g = 1.0 / (1.0 + np.exp(-(beta - ht) * R))
            s = s + (-1.0 - s) * g
        f = ht
        out[t] = s
    return out  # [T, n]


def kernel(h, mesh_points, raw_density, raw_offset, raw_scale, raw_slope):
    from concourse.bass_utils import run_bass_kernel_spmd

    in_maps, dens, h32, d16sum, M = _host_prep(h, mesh_points, raw_density)

    if "prog" not in _PROG_CACHE:
        _PROG_CACHE["prog"] = _build_program()
    nc = _PROG_CACHE["prog"]

    res = run_bass_kernel_spmd(nc, in_maps, list(range(NCORES)))
    zpart = np.zeros(T, np.float64)
    for c in range(NCORES):
        zpart += (res.results[c]["mpart"].astype(np.float64)
                  .sum(axis=0).reshape(T))

    # host tail: the 31 hysterons beyond 8*640
    s_tail = _host_tail_states(h, mesh_points)          # [T, 31]
    num_tail = s_tail @ dens[NDEV:]                     # [T]

    def sigm(x):
        return 1.0 / (1.0 + np.exp(-np.float64(np.asarray(x, np.float32)[0])))

    offset = -10.0 + 20.0 * sigm(raw_offset)
    scale = 20.0 * sigm(raw_scale)
    slope = -20.0 + 40.0 * sigm(raw_slope)

    # s = 2u-1, u = M - z  =>  sum(d*s) = d16sum*(2M-1) - 2*sum(d*z)
    num_dev = d16sum * (2.0 * M - 1.0) - 2.0 * zpart
    m = (num_dev + num_tail) / dens.sum()
    out = scale * m + h32.astype(np.float64) * slope + offset
    return out.astype(np.float32)
